# revision 23
# baseline (speedup 1.0000x reference)
"""MoE routing gate kernel for Trainium2 (8 NeuronCores, data-parallel).

Problem (hardcoded): x [4, 4096, 2048] f32, w_gate [64, 2048] f32,
expert_bias [64] f32 (zeros per spec).
  gate_logits = x @ w_gate.T          # [B, S, 64]
  gate_weights = sigmoid(gate_logits)
  topk_vals, topk_idx = top_k(gate_logits + bias, k=8)
  topk_weights = gather(gate_weights, topk_idx); normalize
Returns (topk_weights [4,4096,8] f32, topk_indices [4,4096,8] int32).

Strategy: shard the 16384 tokens across 8 cores (2048 each); replicate
w_gate. Host pre-packs each core's x slice into a PE-friendly layout
[g, dp, k, tau] = x[token g*512+tau, d = k*128+dp], so the device
kernel streams large contiguous tiles from HBM straight into the
tensor engine's *moving* operand with the small router weight as the
stationary operand. The router weight is only 64 experts wide, so the
kernel runs the two 256-token halves of each group as a concurrent
column-group pair on the PE array (cols 0-63 / 64-127):
  ps[0:64,  t] += wt_k[128,64].T @ x_k[128, tok   0:256]   (cols 0-63)
  ps[64:128,t] += wt_k[128,64].T @ x_k[128, tok 256:512]   (cols 64-127)
which halves the fp32 matmul stream time while keeping the fp32
accumulation bit-identical to the unpaired version. Logits are then
re-transposed token-major via 128x128 PE transposes (the 64:128
partition half transposes against an identity replicated into
partitions 64-127), and per 128-token tile the DVE max/max_index ops
give the top-8 logits+indices. Sigmoid + normalization of the top-8
values happens on the host (monotonic sigmoid keeps the device's
top-8 selection/order identical to the reference's).
"""

import numpy as np

_B, _S, _D, _E = 4, 4096, 2048, 64
_K = 8
_NCORES = 8
_TOK = _B * _S              # 16384 tokens
_TC = _TOK // _NCORES       # 2048 tokens per core
_NG = 4                     # token groups of 512 per core
_GT = 512                   # tokens per group (PSUM bank / fp32 moving max)
_GH = _GT // 2              # tokens per column-group half
_NKC = _D // 128            # 16 contraction chunks

_prog_cache = {}


def _ensure_path():
    import sys
    for p in ("/opt/trn_rl_repo",):
        if p not in sys.path:
            sys.path.insert(0, p)


def _build_program(mode="pair"):
    """Per-core Bass/Tile program (SPMD: same program, different data)."""
    _ensure_path()
    import concourse.bass as bass
    import concourse.tile as tile
    from concourse import bacc, mybir

    nc = bacc.Bacc("TRN2", target_bir_lowering=False, debug=False,
                   num_devices=_NCORES)

    f32 = mybir.dt.float32
    u32 = mybir.dt.uint32

    # DRAM I/O (per core). x layout: [g, dp, k, tau] so each 512-token
    # group streams as one fully-contiguous-per-partition block.
    xg = nc.dram_tensor("xg", [_NG, 128, _NKC, _GT], f32,
                        kind="ExternalInput")
    # wid: k-chunked router weight [128, 16*64] with a 64-col identity
    # block (replicated into both partition halves) appended.
    wid = nc.dram_tensor("wid", [128, _NKC * _E + _E], f32,
                         kind="ExternalInput")
    out_v = nc.dram_tensor("out_v", [128, _NG, _NG, _K], f32,
                           kind="ExternalOutput")
    out_i = nc.dram_tensor("out_i", [128, _NG, _NG, _K], u32,
                           kind="ExternalOutput")

    # k-chunk split per group's DMA: a small opener so the PE starts
    # early, 2MB quanta for stream efficiency (more dispatches measurably
    # slow the stream), finest at the very end so the post-stream
    # dependency chain is one k-chunk.
    subchunks = ((1, 7, 8), (8, 8), (8, 8), (8, 4, 2, 1, 1))

    with tile.TileContext(nc) as tc:
        with (
            tc.tile_pool(name="xpool", bufs=4) as xpool,
            tc.tile_pool(name="wpool", bufs=1) as wpool,
            tc.tile_pool(name="psA", bufs=3, space=bass.MemorySpace.PSUM) as psA,
            tc.tile_pool(name="psB", bufs=2, space=bass.MemorySpace.PSUM) as psB,
            tc.tile_pool(name="lpool", bufs=2) as lpool,
            tc.tile_pool(name="opool", bufs=2) as opool,
        ):
            # All loads ride the sync ring in dependency order (FIFO per
            # ring). The k=0 weight slice goes first so the opening
            # matmul gates only on it plus the first 512 KiB x chunk.
            wt0_sb = wpool.tile([128, _E], f32)
            nc.sync.dma_start(wt0_sb[:], wid[:, 0:_E])
            xt0 = xpool.tile([128, _NKC, _GT], f32, tag="xg")
            nc.sync.dma_start(xt0[:, 0:1, :], xg[0][:, 0:1, :])
            wtR_sb = wpool.tile([128, (_NKC - 1) * _E + _E], f32)
            nc.sync.dma_start(wtR_sb[:], wid[:, _E:])
            idAB = wtR_sb[:, (_NKC - 1) * _E:]          # [128, 64] identity

            def wt_k(k):
                return wt0_sb[:] if k == 0 else wtR_sb[:, bass.ts(k - 1, _E)]

            # PE warm-up: the HAM clock gate keeps the PE at 1.2 GHz until
            # it has seen ~3.4us of sustained matmul activity. Run dummy
            # matmuls on zeroed scratch while the first x chunks stream so
            # the real matmuls start at 2.4 GHz.
            scratch = wpool.tile([128, 320], f32, name="scratch")
            nc.vector.memset(scratch[:], 0)
            psW = psA.tile([128, _GT], f32, tag="psL", name="psWarm")
            for _ in range(7):
                nc.tensor.matmul(
                    psW[0:64, 0:256], scratch[:, 0:64], scratch[:, 64:320],
                    start=True, stop=True,
                )

            xts, psLs, psHs, lgs = {}, {}, {}, {}

            def load_group(g):
                xt = xt0 if g == 0 else xpool.tile([128, _NKC, _GT], f32,
                                                   tag="xg")
                xts[g] = xt
                k0 = 1 if g == 0 else 0
                for nk in subchunks[g]:
                    nk = min(nk, _NKC - k0)
                    nc.sync.dma_start(
                        xt[:, k0:k0 + nk, :],
                        xg[g][:, k0:k0 + nk, :],
                    )
                    k0 += nk

            def mm_group(g, k_lo, k_hi):
                # Column-group pair: token half 0 -> PE cols 0-63 (PSUM
                # partitions 0-63), half 1 -> cols 64-127. Full-bank
                # tiles: each accumulation group must own its 2KB PSUM
                # zero region.
                if k_lo == 0:
                    psLs[g] = psA.tile([128, _GT], f32, tag="psL", name=f"psL{g}")
                    psHs[g] = psA.tile([128, _GT], f32, tag="psH", name=f"psH{g}")
                psL, psH, xt = psLs[g], psHs[g], xts[g]
                for k in range(k_lo, k_hi):
                    nc.tensor.matmul(
                        psL[0:64, 0:_GH], wt_k(k), xt[:, k, 0:_GH],
                        start=(k == 0), stop=(k == _NKC - 1),
                    )
                    nc.tensor.matmul(
                        psH[64:128, 0:_GH], wt_k(k), xt[:, k, _GH:_GT],
                        start=(k == 0), stop=(k == _NKC - 1),
                    )

            def copy_group(g):
                # Both halves PSUM -> SBUF (frees the PSUM banks early).
                lg = lpool.tile([128, _GH], f32, tag="lg", name=f"lg{g}")
                lgs[g] = lg
                nc.scalar.copy(lg[0:64, :], psLs[g][0:64, 0:_GH])
                nc.scalar.copy(lg[64:128, :], psHs[g][64:128, 0:_GH])

            def finish_group(g):
                # 4 PE transposes to token-major [128 tok, 64 expert],
                # then top-8 logits + indices per 128-token tile read
                # straight from PSUM; sigmoid and normalization happen
                # host-side.
                lg = lgs[g]
                ps2 = psB.tile([128, _NG, _E], f32, tag="ps2", name=f"ps2_{g}")
                nc.tensor.transpose(ps2[:, 0, :], lg[0:64, 0:128],
                                    idAB[0:64, :])
                nc.tensor.transpose(ps2[:, 1, :], lg[0:64, 128:256],
                                    idAB[0:64, :])
                nc.tensor.transpose(ps2[:, 2, :], lg[64:128, 0:128],
                                    idAB[64:128, :])
                nc.tensor.transpose(ps2[:, 3, :], lg[64:128, 128:256],
                                    idAB[64:128, :])
                wg = opool.tile([128, _NG, _K], f32, tag="wg", name=f"wg{g}")
                ig = opool.tile([128, _NG, _K], u32, tag="ig", name=f"ig{g}")
                for j in range(_NG):
                    nc.vector.max(wg[:, j, :], ps2[:, j, :])
                    nc.vector.max_index(ig[:, j, :], wg[:, j, :],
                                        ps2[:, j, :])
                    if g == _NG - 1 and j == 1:
                        # Last group: ship the first half early so the
                        # final store only gates on the last two tiles.
                        nc.scalar.dma_start(out_v[:, g, 0:2], wg[:, 0:2])
                        nc.scalar.dma_start(out_i[:, g, 0:2], ig[:, 0:2])
                if g == _NG - 1:
                    nc.scalar.dma_start(out_v[:, g, 2:4], wg[:, 2:4])
                    nc.scalar.dma_start(out_i[:, g, 2:4], ig[:, 2:4])
                else:
                    nc.scalar.dma_start(out_v[:, g], wg[:])
                    nc.scalar.dma_start(out_i[:, g], ig[:])

            # Staged pipeline: each group's transpose/top-8 block is
            # emitted mid-way through the NEXT group's matmul stream so
            # the PE never stalls on the PSUM->SBUF copy, and the end of
            # the program is just the last group's short finish chain.
            for g in range(_NG):
                load_group(g)
            for g in range(_NG):
                mm_group(g, 0, _NKC)
                copy_group(g)
                finish_group(g)

    nc.compile()
    return nc


def _get_program(mode="pair"):
    if mode not in _prog_cache:
        _prog_cache[mode] = _build_program(mode)
    return _prog_cache[mode]


def _pack_inputs(x, w_gate):
    """Host-side layout transform. Returns per-core input maps."""
    x2 = np.ascontiguousarray(x, dtype=np.float32).reshape(_TOK, _D)
    # wt[dp, k*64+e] = w_gate[e, k*128+dp]; identity block appended,
    # replicated into both partition halves for the 64:128 transposes.
    wt = w_gate.T.reshape(_NKC, 128, _E).transpose(1, 0, 2).reshape(
        128, _NKC * _E)
    ident = np.tile(np.eye(_E, dtype=np.float32), (2, 1))
    wid = np.ascontiguousarray(
        np.concatenate([wt, ident], axis=1), dtype=np.float32)
    in_maps = []
    for c in range(_NCORES):
        xc = x2[c * _TC:(c + 1) * _TC]                 # [2048 tok, 2048 d]
        # [g, tau, k, dp] -> [g, dp, k, tau]
        xgc = np.ascontiguousarray(
            xc.reshape(_NG, _GT, _NKC, 128).transpose(0, 3, 2, 1)
        )
        in_maps.append({"xg": xgc, "wid": wid})
    return in_maps


def _unpack_outputs(results):
    v_parts, i_parts = [], []
    for r in results:
        # [128 tau, 4 g, 4 j, 8] -> token (4g+j)*128+tau -> [2048, 8]
        v_parts.append(
            r["out_v"].reshape(128, _NG * _NG, _K).transpose(1, 0, 2).reshape(_TC, _K)
        )
        i_parts.append(
            r["out_i"].reshape(128, _NG * _NG, _K).transpose(1, 0, 2).reshape(_TC, _K)
        )
    vals = np.concatenate(v_parts, axis=0)             # top-8 logits
    indices = (
        np.concatenate(i_parts, axis=0).astype(np.int32).reshape(_B, _S, _K)
    )
    # Host-side epilogue: sigmoid + normalize (sigmoid is monotonic, so
    # the device's top-8 selection order matches the reference's).
    gw = 1.0 / (1.0 + np.exp(-vals, dtype=np.float32))
    weights = (gw / gw.sum(axis=-1, keepdims=True)).astype(
        np.float32).reshape(_B, _S, _K)
    return weights, indices


def _numpy_reference(x, w_gate, expert_bias):
    """Exact fallback for the (unspecced) nonzero-bias case."""
    x2 = np.asarray(x, dtype=np.float32).reshape(_TOK, _D)
    logits = x2 @ np.asarray(w_gate, dtype=np.float32).T
    gw = 1.0 / (1.0 + np.exp(-logits))
    biased = logits + np.asarray(expert_bias, dtype=np.float32)
    idx = np.argsort(-biased, axis=-1, kind="stable")[:, :_K].astype(np.int32)
    tw = np.take_along_axis(gw, idx, axis=-1)
    tw = tw / tw.sum(axis=-1, keepdims=True)
    return (
        tw.reshape(_B, _S, _K).astype(np.float32),
        idx.reshape(_B, _S, _K).astype(np.int32),
    )


def _run(x, w_gate, expert_bias, trace=False, mode="pair", trace_kwargs=None):
    _ensure_path()
    from concourse.bass_utils import run_bass_kernel_spmd

    nc = _get_program(mode)
    in_maps = _pack_inputs(x, w_gate)
    res = run_bass_kernel_spmd(
        nc, in_maps, list(range(_NCORES)), trace=trace,
        **(trace_kwargs or {}),
    )
    weights, indices = _unpack_outputs(res.results)
    return (weights, indices), res


def kernel(x, w_gate, expert_bias):
    x = np.asarray(x)
    w_gate = np.asarray(w_gate)
    expert_bias = np.asarray(expert_bias)
    assert x.shape == (_B, _S, _D), x.shape
    assert w_gate.shape == (_E, _D), w_gate.shape
    if np.any(expert_bias):
        # Spec pins expert_bias to zeros; keep a correct host path anyway.
        return _numpy_reference(x, w_gate, expert_bias)
    try:
        (weights, indices), _ = _run(x, w_gate, expert_bias)
    except Exception:
        # Transient NRT device wedges have been observed on a first
        # execution; one retry has always recovered.
        import time
        time.sleep(10)
        (weights, indices), _ = _run(x, w_gate, expert_bias)
    return weights, indices
